# revision 4
# baseline (speedup 1.0000x reference)
"""Trainium2 Bass kernel for nn_AttentionModule (dense transformer block).

Computes, per batch element:
    x_pe = x + sinusoidal_pos_encoding
    boundary scores s = sigmoid((relu(x_pe @ w1 + b1) @ w2 + b2) / 2)
    bias[i,j] = 0.5*(s_i + s_j)  (the 0.5*s_i row term cancels in softmax)
    Q,K,V projections; scores = QK^T/sqrt(dk) + bias (+ key mask)
    attn = softmax(scores); out = attn @ V; y = LayerNorm(x + out @ wo)
Returns (y, attn).

Sharding: data-parallel over batch — batch b -> NeuronCore b (8 cores).
Each core runs an identical single-core program on its own batch slice;
no collectives are needed.

Per-core dataflow (all fp32):
  - x^T built on-device via PE transposes; x_pe^T = x^T + peT (host const).
  - Q^T/K^T stored d-major ("chunk" layout, 2 heads per 128-partition
    chunk) so pass-1 scores^T (j-major) can pack two DK=64 heads into the
    PE array via row tile_position. V stored s-major with a ones column
    appended per head, so the attn@V matmul also yields the softmax
    denominators (row 64 of the output) for free.
  - pass-1: scores^T[j,i] -> ACT exp (per-partition bias = 0.5*s_j + mask)
    -> unnormalized attnU^T tiles -> attn@V accumulation.
  - attn output needs [i,j] layout. Heads 0..X-1 recompute scores[i,j]
    with an augmented k=65 matmul (ones row x t_row gives the +t_j term)
    and evict through ACT exp with per-partition bias -ln(sum_i), which
    yields normalized attn directly. Heads X..7 instead PE-transpose the
    attnU^T tiles and normalize on DVE with a per-partition 1/sum scale.
    The split balances ACT vs PE/DVE load.
  - out-projection consumes O^T (d-major) after per-head 1/sum scaling
    (broadcast tile built with k=1 outer-product matmuls); LayerNorm via
    bn_stats/bn_aggr.
"""

import math
from contextlib import ExitStack

import numpy as np

B, S, D, H = 8, 1024, 512, 8
DK = D // H          # 64
TEMP = 2.0
EPS = 1e-5
NEG = -1e9
N_CORES = 8
P = 128
NT = S // P          # 8 s-tiles
NC4 = D // P         # 4 d-chunks
NF = 2               # bp hidden chunks (256/128)
X_ACT = 5            # heads 0..4 via ACT-recompute pass-2; 5..7 via PE-transpose


def _pos_encoding_np():
    pos = np.arange(S, dtype=np.float32)[:, None]
    div = np.exp(np.arange(0, D, 2, dtype=np.float32) * (-math.log(10000.0) / D))
    pe = np.zeros((S, D), dtype=np.float32)
    pe[:, 0::2] = np.sin(pos * div)
    pe[:, 1::2] = np.cos(pos * div)
    return pe


def _build(b2_half: float, has_bv: bool, has_bo: bool, has_lnb: bool,
           has_mask: bool):
    import concourse.bass as bass
    import concourse.mybir as mybir
    import concourse.tile as tile
    from concourse import bacc
    from concourse.masks import make_identity

    f32 = mybir.dt.float32
    AF = mybir.ActivationFunctionType
    OP = mybir.AluOpType

    nc = bacc.Bacc("TRN2", target_bir_lowering=False, debug=False,
                   enable_asserts=False, num_devices=N_CORES)

    x_d = nc.dram_tensor("x", [S, D], f32, kind="ExternalInput").ap()
    peT_d = nc.dram_tensor("peT", [D, S], f32, kind="ExternalInput").ap()
    wq_d = nc.dram_tensor("wq", [D, D], f32, kind="ExternalInput").ap()
    wk_d = nc.dram_tensor("wk", [D, D], f32, kind="ExternalInput").ap()
    wv_d = nc.dram_tensor("wv", [D, D], f32, kind="ExternalInput").ap()
    wo_d = nc.dram_tensor("wo", [D, D], f32, kind="ExternalInput").ap()
    bq_d = nc.dram_tensor("bq", [D], f32, kind="ExternalInput").ap()
    bk_d = nc.dram_tensor("bk", [D], f32, kind="ExternalInput").ap()
    bv_d = nc.dram_tensor("bv", [D], f32, kind="ExternalInput").ap()
    bo_d = nc.dram_tensor("bo", [D], f32, kind="ExternalInput").ap()
    w1_d = nc.dram_tensor("w1", [D, D // 2], f32, kind="ExternalInput").ap()
    b1_d = nc.dram_tensor("b1", [D // 2], f32, kind="ExternalInput").ap()
    w2_d = nc.dram_tensor("w2", [D // 2, 1], f32, kind="ExternalInput").ap()
    g_d = nc.dram_tensor("ln_g", [D], f32, kind="ExternalInput").ap()
    lnb_d = nc.dram_tensor("ln_b", [D], f32, kind="ExternalInput").ap()
    mneg_d = nc.dram_tensor("mneg", [S], f32, kind="ExternalInput").ap()

    y_d = nc.dram_tensor("y", [S, D], f32, kind="ExternalOutput").ap()
    attn_d = nc.dram_tensor("attn", [H, S, S], f32, kind="ExternalOutput").ap()

    def bcast(ap_1d, n):
        return bass.AP(tensor=ap_1d.tensor, offset=ap_1d.offset,
                       ap=[[0, P], [1, n]])

    with tile.TileContext(nc) as tc, ExitStack() as ctx:
        const = ctx.enter_context(tc.tile_pool(name="const", bufs=1))
        pers = ctx.enter_context(tc.tile_pool(name="pers", bufs=1))
        psA = ctx.enter_context(tc.tile_pool(name="psA", bufs=2, space="PSUM"))
        psO = ctx.enter_context(tc.tile_pool(name="psO", bufs=4, space="PSUM"))

        # ---------- constants ----------
        ident = const.tile([P, P], f32)
        make_identity(nc, ident)
        ones11 = const.tile([1, 1], f32)
        nc.vector.memset(ones11[:], 1.0)
        e_top = const.tile([1, P], f32)
        nc.vector.memset(e_top[0:1, 0:64], 1.0)
        nc.vector.memset(e_top[0:1, 64:128], 0.0)
        e_bot = const.tile([1, P], f32)
        nc.vector.memset(e_bot[0:1, 0:64], 0.0)
        nc.vector.memset(e_bot[0:1, 64:128], 1.0)
        eps_t = const.tile([P, 1], f32)
        nc.vector.memset(eps_t[:], EPS)
        g_bc = const.tile([P, D], f32)
        nc.sync.dma_start(out=g_bc[:], in_=bcast(g_d, D))
        if has_lnb:
            lnb_bc = const.tile([P, D], f32)
            nc.sync.dma_start(out=lnb_bc[:], in_=bcast(lnb_d, D))
        if has_bv:
            bv_bc = const.tile([P, D], f32)
            nc.sync.dma_start(out=bv_bc[:], in_=bcast(bv_d, D))
        if has_bo:
            bo_bc = const.tile([P, D], f32)
            nc.sync.dma_start(out=bo_bc[:], in_=bcast(bo_d, D))
        bqc = const.tile([P, NC4], f32)
        nc.sync.dma_start(out=bqc[:], in_=bq_d.rearrange("(c p) -> p c", p=P))
        bkc = const.tile([P, NC4], f32)
        nc.sync.dma_start(out=bkc[:], in_=bk_d.rearrange("(c p) -> p c", p=P))
        b1c = const.tile([P, NF], f32)
        nc.sync.dma_start(out=b1c[:], in_=b1_d.rearrange("(c p) -> p c", p=P))
        w2t = const.tile([P, NF, 1], f32)
        nc.sync.dma_start(out=w2t[:], in_=w2_d.rearrange("(c p) o -> p c o", p=P))
        if has_mask:
            mnegc = const.tile([P, NT], f32)
            nc.sync.dma_start(out=mnegc[:],
                              in_=mneg_d.rearrange("(t p) -> p t", p=P))
            mnegr = const.tile([1, S], f32)
            nc.sync.dma_start(
                out=mnegr[:],
                in_=bass.AP(tensor=mneg_d.tensor, offset=mneg_d.offset,
                            ap=[[0, 1], [1, S]]))

        tcols = const.tile([P, NT], f32)          # 0.5*s_b (+mask) per j-tile
        trow = const.tile([1, S], f32)            # same, row layout
        lncols = const.tile([P, X_ACT, NT], f32)  # -ln(sum) cols, ACT heads
        rcols = const.tile([P, H - X_ACT, NT], f32)  # 1/sum cols, transp heads

        # ---------- persistent tensors ----------
        x_sb = pers.tile([P, NT, D], f32)
        nc.sync.dma_start(out=x_sb[:], in_=x_d.rearrange("(t p) d -> p t d", p=P))
        wo_sb = pers.tile([P, NC4, D], f32)
        nc.sync.dma_start(out=wo_sb[:], in_=wo_d.rearrange("(c p) n -> p c n", p=P))
        Qc = pers.tile([P, NC4, S], f32)
        Kc = pers.tile([P, NC4, S], f32)
        Vt = pers.tile([P, NT, H * (DK + 1)], f32)
        OT = pers.tile([P, NC4, S], f32)
        QTa = [pers.tile([DK + 1, S], f32, name=f"qta{h}", tag=f"qta{h}")
               for h in range(X_ACT)]
        KTa = [pers.tile([DK + 1, S], f32, name=f"kta{h}", tag=f"kta{h}")
               for h in range(X_ACT)]

        # ---------- stages 1-5 (scoped working set) ----------
        with tc.tile_pool(name="s15", bufs=1) as s15:
            xpeT = s15.tile([P, NC4, S], f32, tag="xpeT")
            hT = s15.tile([P, NF, S], f32, tag="hT")

            # stage 1: x^T via PE transpose, + peT -> x_pe^T
            for c in range(NC4):
                peT_sb = s15.tile([P, S], f32, name=f"peT{c}", tag="peT", bufs=2)
                nc.sync.dma_start(out=peT_sb[:], in_=peT_d[c * P:(c + 1) * P, :])
                for q in range(2):
                    pt = psA.tile([P, 2 * 512], f32, tag="A")
                    for t4 in range(4):
                        t = q * 4 + t4
                        nc.tensor.matmul(
                            pt[:, t4 * P:(t4 + 1) * P],
                            lhsT=x_sb[:, t, c * P:(c + 1) * P],
                            rhs=ident[:], start=True, stop=True)
                    nc.any.tensor_add(xpeT[:, c, q * 512:(q + 1) * 512],
                                      pt[:, 0:512],
                                      peT_sb[:, q * 512:(q + 1) * 512])

            # stage 2: hT = relu(w1^T x_pe^T + b1)
            w1_sb = s15.tile([P, NC4, D // 2], f32, tag="w1")
            nc.sync.dma_start(out=w1_sb[:],
                              in_=w1_d.rearrange("(c p) n -> p c n", p=P))
            for m in range(NF):
                for n in range(2):
                    ph = psA.tile([P, 2 * 512], f32, tag="A")
                    for k in range(NC4):
                        nc.tensor.matmul(
                            ph[:, 0:512],
                            lhsT=w1_sb[:, k, m * P:(m + 1) * P],
                            rhs=xpeT[:, k, n * 512:(n + 1) * 512],
                            start=(k == 0), stop=(k == NC4 - 1))
                    nc.scalar.activation(hT[:, m, n * 512:(n + 1) * 512],
                                         ph[:, 0:512], AF.Relu,
                                         bias=b1c[:, m:m + 1], scale=1.0)

            # stage 3: t_cols = 0.5*sigmoid(s_raw/2) (+mask)
            for t in range(NT):
                ps1 = psA.tile([P, 2 * 512], f32, tag="A")
                for k2 in range(NF):
                    nc.tensor.matmul(ps1[:, 0:1],
                                     lhsT=hT[:, k2, t * P:(t + 1) * P],
                                     rhs=w2t[:, k2, :],
                                     start=(k2 == 0), stop=(k2 == NF - 1))
                esb = s15.tile([P, 1], f32, name=f"sig{t}", tag="sig", bufs=2)
                nc.scalar.activation(esb[:], ps1[:, 0:1], AF.Exp,
                                     bias=-b2_half, scale=-0.5)
                nc.vector.tensor_scalar_add(esb[:], esb[:], 1.0)
                nc.vector.reciprocal(esb[:], esb[:])
                if has_mask:
                    nc.vector.tensor_scalar(tcols[:, t:t + 1], esb[:], 0.5,
                                            None, OP.mult)
                    nc.vector.tensor_add(tcols[:, t:t + 1], tcols[:, t:t + 1],
                                         mnegc[:, t:t + 1])
                else:
                    nc.vector.tensor_scalar_mul(tcols[:, t:t + 1], esb[:], 0.5)

            # stage 4: t_row (same values, [1,S] layout)
            for n in range(2):
                ps2 = psA.tile([P, 2 * 512], f32, tag="A")
                for k2 in range(NF):
                    nc.tensor.matmul(ps2[0:1, 0:512],
                                     lhsT=w2t[:, k2, :],
                                     rhs=hT[:, k2, n * 512:(n + 1) * 512],
                                     start=(k2 == 0), stop=(k2 == NF - 1))
                esr = s15.tile([1, 512], f32, name=f"sigr{n}", tag="sigr", bufs=2)
                nc.scalar.activation(esr[:], ps2[0:1, 0:512], AF.Exp,
                                     bias=-b2_half, scale=-0.5)
                nc.vector.tensor_scalar_add(esr[:], esr[:], 1.0)
                nc.vector.reciprocal(esr[:], esr[:])
                nc.vector.tensor_scalar_mul(esr[:], esr[:], 0.5)
                if has_mask:
                    nc.vector.tensor_add(trow[0:1, n * 512:(n + 1) * 512],
                                         esr[:], mnegr[0:1, n * 512:(n + 1) * 512])
                else:
                    nc.vector.tensor_copy(trow[0:1, n * 512:(n + 1) * 512],
                                          esr[:])

            # stage 5: projections (weights streamed through one shared tag)
            def proj_chunked(w_sb, bias_col, Cc, Ca):
                """Q^T/K^T chunk-layout projection (+per-head aug evicts)."""
                for c in range(NC4):
                    for n in range(2):
                        sl = slice(n * 512, (n + 1) * 512)
                        pq = psA.tile([P, 2 * 512], f32, tag="A")
                        for k in range(NC4):
                            nc.tensor.matmul(pq[:, 0:512],
                                             lhsT=w_sb[:, k, c * P:(c + 1) * P],
                                             rhs=xpeT[:, k, sl],
                                             start=(k == 0), stop=(k == NC4 - 1))
                        nc.any.tensor_scalar_add(Cc[:, c, sl], pq[:, 0:512],
                                                 bias_col[:, c:c + 1])
                        for h in (2 * c, 2 * c + 1):
                            if h < X_ACT:
                                hb = (h % 2) * 64
                                nc.any.tensor_scalar_add(
                                    Ca[h][0:64, sl], pq[hb:hb + 64, 0:512],
                                    bias_col[hb:hb + 64, c:c + 1])

            wq_sb = s15.tile([P, NC4, D], f32, name="wq_sb", tag="wproj", bufs=2)
            nc.sync.dma_start(out=wq_sb[:],
                              in_=wq_d.rearrange("(c p) n -> p c n", p=P))
            proj_chunked(wq_sb, bqc, Qc, QTa)
            wk_sb = s15.tile([P, NC4, D], f32, name="wk_sb", tag="wproj", bufs=2)
            nc.sync.dma_start(out=wk_sb[:],
                              in_=wk_d.rearrange("(c p) n -> p c n", p=P))
            proj_chunked(wk_sb, bkc, Kc, KTa)

            wv_sb = s15.tile([P, NC4, D], f32, name="wv_sb", tag="wproj", bufs=2)
            nc.sync.dma_start(out=wv_sb[:],
                              in_=wv_d.rearrange("(c p) n -> p c n", p=P))
            for t in range(NT):       # V (s-major) with ones column per head
                pv = psA.tile([P, 2 * 512], f32, tag="A")
                for k in range(NC4):
                    nc.tensor.matmul(pv[:, 0:512],
                                     lhsT=xpeT[:, k, t * P:(t + 1) * P],
                                     rhs=wv_sb[:, k, :],
                                     start=(k == 0), stop=(k == NC4 - 1))
                v_out = Vt[:, t, :].rearrange("p (h c) -> p h c", c=DK + 1)
                pv_in = pv[:, 0:512].rearrange("p (h c) -> p h c", c=DK)
                if has_bv:
                    nc.any.tensor_add(
                        v_out[:, :, 0:DK], pv_in,
                        bv_bc[:].rearrange("p (h c) -> p h c", c=DK))
                else:
                    nc.any.tensor_copy(v_out[:, :, 0:DK], pv_in)
            nc.vector.memset(
                Vt[:].rearrange("p t (h c) -> p t h c", c=DK + 1)[:, :, :, DK:DK + 1],
                1.0)
            for h in range(X_ACT):
                nc.vector.memset(QTa[h][64:65, :], 1.0)
                nc.any.tensor_copy(KTa[h][64:65, :], trow[:])

        # ---------- stage 6: attention ----------
        attnU = ctx.enter_context(tc.tile_pool(name="attnU", bufs=8))
        aout = ctx.enter_context(tc.tile_pool(name="aout", bufs=2))
        rr = ctx.enter_context(tc.tile_pool(name="rr", bufs=3))
        yst = ctx.enter_context(tc.tile_pool(name="yst", bufs=2))

        def pass1_evict(h, J, ps):
            """exp(scores^T + t_j) -> attnU tile [128 j, 1024 i]."""
            t = attnU.tile([P, S], f32, name=f"aU{h}_{J}", tag="attnU")
            nc.scalar.activation(t[:], ps[:], AF.Exp,
                                 bias=tcols[:, J:J + 1], scale=1.0)
            return t

        def av_step(h, J, aU, psAV):
            vs = slice(h * (DK + 1), (h + 1) * (DK + 1))
            for n in range(2):
                nc.tensor.matmul(psAV[n][:],
                                 lhsT=Vt[:, J, vs],
                                 rhs=aU[:, n * 512:(n + 1) * 512],
                                 start=(J == 0), stop=(J == NT - 1))

        def phase_b(h, psAV):
            """O^T evict, sums -> rrow; lncols (ACT) / rcols (transpose)."""
            c, hb = h // 2, (h % 2) * 64
            rrow = rr.tile([1, S], f32, name=f"rrow{h}", tag="rr")
            for n in range(2):
                sl = slice(n * 512, (n + 1) * 512)
                nc.any.tensor_copy(OT[hb:hb + 64, c, sl], psAV[n][0:64, :])
                nc.vector.reciprocal(rrow[0:1, sl], psAV[n][64:65, :])
            pc = psA.tile([P, 2 * 512], f32, tag="A")
            for I in range(NT):
                nc.tensor.matmul(pc[:, I:I + 1],
                                 lhsT=rrow[0:1, I * P:(I + 1) * P],
                                 rhs=ones11[:], start=True, stop=True)
            if h < X_ACT:
                nc.scalar.activation(lncols[:, h, :], pc[:, 0:NT], AF.Ln,
                                     bias=0.0, scale=1.0)
            else:
                nc.vector.tensor_copy(rcols[:, h - X_ACT, :], pc[:, 0:NT])
            return rrow

        def chunk_normalize(c, rrow_a, rrow_b):
            """OT[:, c, :] *= per-head 1/sum broadcast (outer products)."""
            prb = psA.tile([P, 2 * 512], f32, tag="A")
            for n in range(2):
                sl = slice(n * 512, (n + 1) * 512)
                nc.tensor.matmul(prb[:, sl], lhsT=e_top[:],
                                 rhs=rrow_a[0:1, sl], start=True, stop=False)
                nc.tensor.matmul(prb[:, sl], lhsT=e_bot[:],
                                 rhs=rrow_b[0:1, sl], start=False, stop=True)
            nc.vector.tensor_mul(OT[:, c, :], OT[:, c, :], prb[:])

        def phase_c_act(h):
            """pass-2 via augmented k=65 matmul + exp(-lnSum) eviction."""
            for I in range(NT):
                pp = psA.tile([P, 2 * 512], f32, tag="A")
                for n in range(2):
                    nc.tensor.matmul(pp[:, n * 512:(n + 1) * 512],
                                     lhsT=QTa[h][:, I * P:(I + 1) * P],
                                     rhs=KTa[h][:, n * 512:(n + 1) * 512],
                                     start=True, stop=True)
                at = aout.tile([P, S], f32, name=f"ao{h}_{I}", tag="aout")
                nc.scalar.activation(at[:], pp[:], AF.Exp,
                                     bias=lncols[:, h, I:I + 1], scale=1.0)
                nc.sync.dma_start(out=attn_d[h, I * P:(I + 1) * P, :], in_=at[:])

        def phase_c_transpose(h, aU_tiles):
            """pass-2 via PE transpose of attnU^T + DVE 1/sum scaling."""
            for I in range(NT):
                pp = psA.tile([P, 2 * 512], f32, tag="A")
                for J in range(NT):
                    nc.tensor.matmul(pp[:, J * P:(J + 1) * P],
                                     lhsT=aU_tiles[J][:, I * P:(I + 1) * P],
                                     rhs=ident[:], start=True, stop=True)
                at = aout.tile([P, S], f32, name=f"ao{h}_{I}", tag="aout")
                nc.vector.tensor_scalar_mul(at[:], pp[:],
                                            rcols[:, h - X_ACT, I:I + 1])
                nc.sync.dma_start(out=attn_d[h, I * P:(I + 1) * P, :], in_=at[:])

        # packed pairs (heads 0..3): two k=64 matmuls share the PE array
        for c in range(2):
            hA, hB = 2 * c, 2 * c + 1
            psAV_A = [psO.tile([DK + 1, 512], f32, name=f"psav_a{c}_{i}",
                               tag="psO") for i in range(2)]
            psAV_B = [psO.tile([DK + 1, 512], f32, name=f"psav_b{c}_{i}",
                               tag="psO") for i in range(2)]
            for J in range(NT):
                js = slice(J * P, (J + 1) * P)
                psc_A = psA.tile([P, 2 * 512], f32, tag="A")
                psc_B = psA.tile([P, 2 * 512], f32, tag="A")
                for n in range(2):
                    sl = slice(n * 512, (n + 1) * 512)
                    nc.tensor.matmul(psc_A[:, sl], lhsT=Kc[0:64, c, js],
                                     rhs=Qc[0:64, c, sl],
                                     start=True, stop=True,
                                     tile_position=(0, 0))
                    nc.tensor.matmul(psc_B[:, sl], lhsT=Kc[64:128, c, js],
                                     rhs=Qc[64:128, c, sl],
                                     start=True, stop=True,
                                     tile_position=(64, 0))
                aU_A = pass1_evict(hA, J, psc_A)
                aU_B = pass1_evict(hB, J, psc_B)
                av_step(hA, J, aU_A, psAV_A)
                av_step(hB, J, aU_B, psAV_B)
            rrow_a = phase_b(hA, psAV_A)
            rrow_b = phase_b(hB, psAV_B)
            chunk_normalize(c, rrow_a, rrow_b)
            phase_c_act(hA)
            phase_c_act(hB)

        # heads 4..7: unpacked pass-1
        rrow_prev = None
        for h in range(4, H):
            c, hb = h // 2, (h % 2) * 64
            psAV = [psO.tile([DK + 1, 512], f32, name=f"psav{h}_{i}", tag="psO")
                    for i in range(2)]
            aU_tiles = []
            for J in range(NT):
                js = slice(J * P, (J + 1) * P)
                psc = psA.tile([P, 2 * 512], f32, tag="A")
                for n in range(2):
                    sl = slice(n * 512, (n + 1) * 512)
                    nc.tensor.matmul(psc[:, sl], lhsT=Kc[hb:hb + 64, c, js],
                                     rhs=Qc[hb:hb + 64, c, sl],
                                     start=True, stop=True)
                aU = pass1_evict(h, J, psc)
                av_step(h, J, aU, psAV)
                aU_tiles.append(aU)
            rrow = phase_b(h, psAV)
            if h % 2 == 0:
                rrow_prev = rrow
            else:
                chunk_normalize(c, rrow_prev, rrow)
            if h < X_ACT:
                phase_c_act(h)
            else:
                phase_c_transpose(h, aU_tiles)

        # ---------- stage 7: out-projection + layernorm ----------
        for I in range(NT):
            py = psA.tile([P, 2 * 512], f32, tag="A")
            for c in range(NC4):
                nc.tensor.matmul(py[:, 0:512],
                                 lhsT=OT[:, c, I * P:(I + 1) * P],
                                 rhs=wo_sb[:, c, :],
                                 start=(c == 0), stop=(c == NC4 - 1))
            y0 = yst.tile([P, D], f32, tag="y0")
            nc.any.tensor_add(y0[:], py[:, 0:512], x_sb[:, I, :])
            if has_bo:
                nc.any.tensor_add(y0[:], y0[:], bo_bc[:])
            stats = yst.tile([P, 6], f32, tag="st")
            nc.vector.bn_stats(out=stats[:], in_=y0[:])
            mv = yst.tile([P, 2], f32, tag="mv")
            nc.vector.bn_aggr(out=mv[:], in_=stats[:])
            rstd = yst.tile([P, 1], f32, tag="rs")
            nc.scalar.activation(rstd[:], mv[:, 1:2], AF.Sqrt,
                                 bias=eps_t[:], scale=1.0)
            nc.vector.reciprocal(rstd[:], rstd[:])
            nc.vector.tensor_scalar(y0[:], y0[:], mv[:, 0:1], rstd[:],
                                    OP.subtract, OP.mult)
            nc.vector.tensor_mul(y0[:], y0[:], g_bc[:])
            if has_lnb:
                nc.vector.tensor_add(y0[:], y0[:], lnb_bc[:])
            nc.sync.dma_start(out=y_d[I * P:(I + 1) * P, :], in_=y0[:])

    nc.compile()
    return nc


_NC_CACHE = {}


def _get_nc(key):
    if key not in _NC_CACHE:
        _NC_CACHE[key] = _build(*key)
    return _NC_CACHE[key]


def kernel(x, mask, wq, bq, wk, bk, wv, bv, wo, bo,
           bp_w1, bp_b1, bp_w2, bp_b2, ln_g, ln_b):
    from concourse.bass_utils import run_bass_kernel_spmd

    f = np.float32
    x = np.ascontiguousarray(x, dtype=f)
    mask = np.asarray(mask, dtype=bool)
    b2_half = float(np.asarray(bp_b2, dtype=f).reshape(-1)[0]) * 0.5
    key = (b2_half,
           bool(np.any(np.asarray(bv) != 0)),
           bool(np.any(np.asarray(bo) != 0)),
           bool(np.any(np.asarray(ln_b) != 0)),
           bool(np.any(~mask)))
    nc = _get_nc(key)

    peT = np.ascontiguousarray(_pos_encoding_np().T)
    common = {
        "peT": peT,
        "wq": np.ascontiguousarray(wq, dtype=f) * f(0.125),
        "bq": np.ascontiguousarray(bq, dtype=f) * f(0.125),
        "wk": np.ascontiguousarray(wk, dtype=f),
        "bk": np.ascontiguousarray(bk, dtype=f),
        "wv": np.ascontiguousarray(wv, dtype=f),
        "bv": np.ascontiguousarray(bv, dtype=f),
        "wo": np.ascontiguousarray(wo, dtype=f),
        "bo": np.ascontiguousarray(bo, dtype=f),
        "w1": np.ascontiguousarray(bp_w1, dtype=f),
        "b1": np.ascontiguousarray(bp_b1, dtype=f),
        "w2": np.ascontiguousarray(bp_w2, dtype=f),
        "ln_g": np.ascontiguousarray(ln_g, dtype=f),
        "ln_b": np.ascontiguousarray(ln_b, dtype=f),
    }
    in_maps = []
    for b in range(N_CORES):
        mneg = np.where(mask[b], f(0.0), f(NEG)).astype(f)
        in_maps.append({"x": np.ascontiguousarray(x[b]), "mneg": mneg, **common})

    res = run_bass_kernel_spmd(nc, in_maps, list(range(N_CORES)))
    y = np.stack([res.results[b]["y"] for b in range(N_CORES)])
    attn = np.stack([res.results[b]["attn"] for b in range(N_CORES)])
    return y, attn
